# revision 6
# baseline (speedup 1.0000x reference)
"""Trainium2 Bass kernel for nn_CustomLoss_45449343926664 (retrieval_knn).

loss = mse(mean(c1), mean(c2))
     + mean_i min_j ||c1_i - c2_j||^2
     + mean_k relu(0.1 - var(c1)_k)

Device does ONLY the O(N^2) part: each core computes its [1024, 8192]
block of 2<c1_i, c2_j> on the PE (bf16, j-on-partitions) and row-maxes
z = 2<c1,c2> - |c2_j|^2 with a two-engine drain (DVE + ACT are the only
engines with a PSUM read port).

Key trick: c2 is sorted by |c2_j|^2 on the host and laid out so that
the two j's mapped to (pair s, partition p) hold adjacent sorted values
(j = t*128+p <- sorted[p*64+t]).  Consecutive j-tile PAIRS then share
one per-partition bias vector (error <= the adjacent-gap ~0.05, vs the
~0.5 bf16 quantization already present), so both drain engines process
TWO j-tiles per instruction from a 4-bank psum pair tile:

  - D pairs: one fused scalar_tensor_tensor
        zD' = max(psum[128,2,1024] + bias, zD)   (ping-pong accum)
  - A pairs: one activation(Identity, bias) -> zt [128,2,1024] bf16,
    folded into the zAcc running max by one bf16 tensor_max (2x) on DVE.

PSUM: two pair tiles (8 banks) double-buffered.  Filler matmuls target
the pair about to be rewritten (start=True resets), keeping the PE
busy fraction high so its p-state stays fast.  The final partial-max
tensor zfin ([128, 1024] bf16 per core) is DMA'd out in two halves; the
host does the partition-max, |c1_i|^2, means/variances, and the scalar
combine (all O(N*D); max over j is permutation-invariant so the sort
needs no undo).
"""
import os
import sys

import numpy as np
import ml_dtypes

if os.path.isdir("/opt/trn_rl_repo") and "/opt/trn_rl_repo" not in sys.path:
    sys.path.insert(0, "/opt/trn_rl_repo")

from contextlib import ExitStack

import concourse.bass as bass
import concourse.tile as tile
from concourse import bacc, mybir
from concourse.bass_utils import run_bass_kernel_spmd

F32 = mybir.dt.float32
BF16 = mybir.dt.bfloat16
BF16_NP = ml_dtypes.bfloat16
NEG_BIG = -3.0e38

N_CORES = 8
N1 = 8192            # cluster1 rows (total)
N2 = 8192            # cluster2 rows
D = 128              # feature dim = partition count
P = 128
NI = N1 // N_CORES   # 1024 c1 rows per core
NJT = N2 // P        # 64 j-tiles of 128
NPAIR = NJT // 2     # 32 j-tile pairs

# c2bT DMA chunk sizes in j-tiles (first small so matmuls start early)
CHUNK_JT = [2, 6, 8, 8, 8, 8, 8, 8, 8]
CHUNK_START = [0, 2, 8, 16, 24, 32, 40, 48, 56]
TILE_CHUNK = {}
for _ci, (_s, _n) in enumerate(zip(CHUNK_START, CHUNK_JT)):
    for _k in range(_n):
        TILE_CHUNK[_s + _k] = (_ci, _k)

# 10 pairs drained by the fused DVE path; 22 by ACT (faster clock),
# folded on DVE in bf16 2x mode.  Last pair on DVE so ACT+folds finish
# early and the finale overlaps the tail.
D_PAIR_SLOTS = {2, 5, 8, 11, 14, 17, 20, 23, 27, 31}
N_WARM = 16
FILLER_COLS = 512            # one filler matmul per psum half-tile
MIN_VARIANCE = 0.1

_cached = {}


def _build_program():
    """Build + compile the single-core SPMD program (same for all cores)."""
    nc = bacc.Bacc(
        "TRN2",
        target_bir_lowering=False,
        debug=False,
        enable_asserts=False,
        num_devices=N_CORES,
    )

    d_c1bT = nc.dram_tensor("c1bT", [D, NI], BF16, kind="ExternalInput").ap()
    d_c2bT = nc.dram_tensor("c2bT", [D, N2], BF16, kind="ExternalInput").ap()
    d_sq2p = nc.dram_tensor("sq2p", [P, NPAIR], F32, kind="ExternalInput").ap()

    d_zfin = nc.dram_tensor("zfin", [P, NI], BF16, kind="ExternalOutput").ap()

    with tile.TileContext(nc) as tc, ExitStack() as ctx:
        const = ctx.enter_context(tc.tile_pool(name="const", bufs=1))
        c2pool = ctx.enter_context(tc.tile_pool(name="c2pool", bufs=len(CHUNK_JT)))
        zring = ctx.enter_context(tc.tile_pool(name="zring", bufs=10))
        psumc = ctx.enter_context(tc.tile_pool(name="psumc", bufs=2, space="PSUM"))

        t_c1bT = const.tile([P, NI], BF16)
        t_sq2p = const.tile([P, NPAIR], F32)
        t_warm = const.tile([P, P], BF16)
        t_wact = const.tile([P, P], BF16)
        t_zD = const.tile([P, 2, 2, NI], BF16)     # DVE STT ping-pong pairs
        t_zAcc = const.tile([P, 2, 2, NI], BF16)   # fold-chain ping-pong
        t_zfin = const.tile([P, NI], BF16)

        # ---- input DMAs first (sync + gpsimd queues; ACT/DVE stay clean) ----
        t_c2bT = []
        for ci, (s, n) in enumerate(zip(CHUNK_START, CHUNK_JT)):
            t_c2bT.append(c2pool.tile([P, n, P], BF16, name=f"c2bT{ci}"))
        nc.sync.dma_start(
            t_c2bT[0][:],
            d_c2bT[:, : CHUNK_JT[0] * P].rearrange("k (t p) -> k t p", p=P),
        )
        nc.gpsimd.dma_start(t_c1bT[:], d_c1bT)
        nc.sync.dma_start(t_sq2p[:], d_sq2p)
        dma_engs = [nc.gpsimd, nc.sync]
        for ci in range(1, len(CHUNK_JT)):
            s, n = CHUNK_START[ci], CHUNK_JT[ci]
            dma_engs[ci % 2].dma_start(
                t_c2bT[ci][:],
                d_c2bT[:, s * P : (s + n) * P].rearrange("k (t p) -> k t p", p=P),
            )

        # accumulator init + PE warm operand
        nc.vector.memset(t_warm[:], 0.0)
        nc.gpsimd.memset(t_zD[:, 0], NEG_BIG)
        nc.gpsimd.memset(t_zAcc[:, 0], NEG_BIG)

        # warm the ACT table before the first drain needs it
        nc.scalar.activation(t_wact[:], t_warm[:],
                             mybir.ActivationFunctionType.Identity, bias=0.0)

        # ---- cross matmuls (j on partitions) + two-engine pair drain ----
        nd = nacc = 0
        first = True
        for s in range(NPAIR):
            t0, t1 = 2 * s, 2 * s + 1
            pt = psumc.tile([P, 2, NI], F32, name="pcross")
            if first:
                # PE warm-up into the to-be-reset pair (starts p-state ramp
                # while inputs stream in; start=True below wipes it)
                for _ in range(N_WARM):
                    nc.tensor.matmul(pt[:, 0, :P], t_warm[:], t_warm[:],
                                     start=True, stop=True)
                first = False
            for h, t in enumerate((t0, t1)):
                ci, ck = TILE_CHUNK[t]
                lhsT = t_c2bT[ci][:, ck]
                # filler: keeps PE busy fraction high (overwritten by the
                # start=True matmuls right after)
                nc.tensor.matmul(pt[:, h, :FILLER_COLS], lhsT,
                                 t_c1bT[:, :FILLER_COLS],
                                 start=True, stop=True)
                nc.tensor.matmul(pt[:, h, :512], lhsT, t_c1bT[:, :512],
                                 start=True, stop=True)
                nc.tensor.matmul(pt[:, h, 512:], lhsT, t_c1bT[:, 512:],
                                 start=True, stop=True)
            bias = t_sq2p[:, s : s + 1]
            if s in D_PAIR_SLOTS:
                nc.vector.scalar_tensor_tensor(
                    out=t_zD[:, (nd + 1) % 2],
                    in0=pt[:],
                    scalar=bias,
                    in1=t_zD[:, nd % 2],
                    op0=mybir.AluOpType.add,
                    op1=mybir.AluOpType.max,
                )
                nd += 1
            else:
                zt = zring.tile([P, 2, NI], BF16, name="zt")
                nc.scalar.activation(
                    zt[:], pt[:], mybir.ActivationFunctionType.Identity,
                    bias=bias, scale=1.0,
                )
                nc.vector.tensor_max(t_zAcc[:, (nacc + 1) % 2],
                                     t_zAcc[:, nacc % 2], zt[:])
                nacc += 1

        # ---- final: combine accumulators + ship out (split DMA) ----
        nc.vector.tensor_max(t_zfin[:], t_zAcc[:, nacc % 2, 0],
                             t_zAcc[:, nacc % 2, 1])
        nc.vector.tensor_max(t_zfin[:], t_zfin[:], t_zD[:, nd % 2, 0])
        nc.vector.tensor_max(t_zfin[:], t_zfin[:], t_zD[:, nd % 2, 1])
        nc.sync.dma_start(d_zfin[:, : NI // 2], t_zfin[:, : NI // 2])
        nc.gpsimd.dma_start(d_zfin[:, NI // 2 :], t_zfin[:, NI // 2 :])

    nc.compile()
    return nc


def _prep_inputs(cluster1: np.ndarray, cluster2: np.ndarray):
    """Host-side sharding, |c2|^2-sort, and operand layout prep.

    c2 is permuted so that j = t*128 + p holds sorted-by-|c2|^2 index
    p*64 + t: the two members of any tile pair are sorted-adjacent per
    partition, so a pair shares one bias vector.  max over j is
    permutation-invariant, so nothing needs undoing downstream."""
    c2b = cluster2.astype(BF16_NP)
    sq2 = (c2b.astype(np.float32) ** 2).sum(axis=1)          # [8192] fp32
    order = np.argsort(sq2, kind="stable")
    c2s = c2b[order]                                          # sorted rows
    # [k, sorted] -> [k, (t p)] with sorted = p*64 + t
    c2bT = np.ascontiguousarray(
        c2s.T.reshape(D, P, NJT).transpose(0, 2, 1).reshape(D, N2)
    )
    sq2s = sq2[order].reshape(P, NJT)                         # [p, t]
    sq2pair = -0.5 * (sq2s[:, 0::2] + sq2s[:, 1::2])          # [128, 32]
    sq2pair = np.ascontiguousarray(sq2pair.astype(np.float32))

    in_maps = []
    for c in range(N_CORES):
        c1s = cluster1[c * NI : (c + 1) * NI]
        c1bT = np.ascontiguousarray((2.0 * c1s).astype(BF16_NP).T)  # [128, 1024]
        in_maps.append({
            "c1bT": c1bT,
            "c2bT": c2bT,
            "sq2p": sq2pair,
        })
    return in_maps


def _finish(results, cluster1, cluster2) -> np.float32:
    """Host: partition-max of the per-core partials + the O(N*D) stats."""
    c1 = np.asarray(cluster1, np.float32)
    c2 = np.asarray(cluster2, np.float32)
    dist_sum = 0.0
    for c, r in enumerate(results):
        z = np.asarray(r["zfin"], np.float32)   # [128 j-lane, 1024 i]
        gmax = z.max(axis=0)                    # [1024] max_j (2<c1,c2> - |c2|^2)
        c1s = c1[c * NI : (c + 1) * NI].astype(np.float64)
        sq1 = (c1s ** 2).sum(axis=1)            # [1024]
        dist_sum += (sq1 - gmax.astype(np.float64)).sum()
    dist = dist_sum / N1

    m1 = c1.mean(axis=0, dtype=np.float64)
    m2 = c2.mean(axis=0, dtype=np.float64)
    mean_loss = ((m1 - m2) ** 2).mean()
    q1 = (c1.astype(np.float64) ** 2).mean(axis=0)
    var = q1 - m1 ** 2
    disp = np.maximum(MIN_VARIANCE - var, 0.0).mean()
    return np.float32(mean_loss + dist + disp)


def _run(inputs, trace=False, **kwargs):
    """Run on the 8 NeuronCores. Returns (loss_scalar, BassKernelResults)."""
    if "nc" not in _cached:
        _cached["nc"] = _build_program()
    nc = _cached["nc"]
    c1 = np.asarray(inputs["cluster1"], np.float32)
    c2 = np.asarray(inputs["cluster2"], np.float32)
    in_maps = _prep_inputs(c1, c2)
    res = run_bass_kernel_spmd(nc, in_maps, list(range(N_CORES)), trace=trace,
                               **kwargs)
    loss = _finish(res.results, c1, c2)
    return loss, res


def kernel(cluster1: np.ndarray, cluster2: np.ndarray) -> np.ndarray:
    loss, _ = _run({"cluster1": cluster1, "cluster2": cluster2})
    return np.asarray(loss, dtype=np.float32)


# revision 10
# speedup vs baseline: 1.0375x; 1.0375x over previous
"""Trainium2 Bass kernel for nn_CustomLoss_45449343926664 (retrieval_knn).

loss = mse(mean(c1), mean(c2))
     + mean_i min_j ||c1_i - c2_j||^2
     + mean_k relu(0.1 - var(c1)_k)

Device does ONLY the O(N^2) part: each core computes its [1024, 8192]
block of 2<c1_i, c2_j> on the PE (bf16, j-on-partitions: psum tile
[128 j, 1024 i] per j-tile) and row-maxes z = 2<c1,c2> - |c2_j|^2 with a
two-engine drain (each psum element passes exactly once through DVE or
ACT, the only engines with a PSUM read port; both read PSUM at 1
elem/lane/cycle, which is the hard wall of this kernel):

  - DVE tiles (16): fused scalar_tensor_tensor drain
        zD' = max(psum + bias, zD)     (ping-pong accum)
  - ACT tiles (48): activation(Identity, bias) -> bf16 z quads, folded
    into the zAcc running max by one [128,4,1024] bf16 tensor_max (2x
    mode) on DVE per 4 tiles.

The last 4 j-tiles go to DVE so ACT and the fold chain finish early:
the zAcc finale overlaps the trailing STTs and the tail is one
tensor_max + a split (2-queue) DMA of zfin.  Filler matmuls into a
scratch psum bank keep the PE busy fraction high so its p-state stays
fast.  The host does the partition-max of zfin, |c1_i|^2, the
means/variances, and the scalar combine (all O(N*D))."""
import os
import sys

import numpy as np
import ml_dtypes

if os.path.isdir("/opt/trn_rl_repo") and "/opt/trn_rl_repo" not in sys.path:
    sys.path.insert(0, "/opt/trn_rl_repo")

from contextlib import ExitStack

import concourse.bass as bass
import concourse.tile as tile
from concourse import bacc, mybir
from concourse.bass_utils import run_bass_kernel_spmd

F32 = mybir.dt.float32
BF16 = mybir.dt.bfloat16
BF16_NP = ml_dtypes.bfloat16
NEG_BIG = -3.0e38

N_CORES = 8
N1 = 8192            # cluster1 rows (total)
N2 = 8192            # cluster2 rows
D = 128              # feature dim = partition count
P = 128
NI = N1 // N_CORES   # 1024 c1 rows per core
NJT = N2 // P        # 64 j-tiles of 128

# c2bT DMA chunk sizes in j-tiles (first small so matmuls start early)
CHUNK_JT = [2, 6, 8, 8, 8, 8, 8, 8, 8]
CHUNK_START = [0, 2, 8, 16, 24, 32, 40, 48, 56]
TILE_CHUNK = {}
for _ci, (_s, _n) in enumerate(zip(CHUNK_START, CHUNK_JT)):
    for _k in range(_n):
        TILE_CHUNK[_s + _k] = (_ci, _k)

# 16 tiles drained by the fused DVE path (12 interleaved + the last 4);
# the other 48 go to ACT (whose clock is faster) and are folded on DVE
# in bf16 2x mode, 4 tiles per fold.
DVE_TILES = ({5 * k + 2 for k in range(12)} | {60, 61, 62, 63})
FOLD_W = 4           # ACT z tiles per fold instruction
N_WARM = 16
FILLER_COLS = [512, 128]     # filler matmul widths per j-tile
MIN_VARIANCE = 0.1

_cached = {}


def _build_program():
    """Build + compile the single-core SPMD program (same for all cores)."""
    nc = bacc.Bacc(
        "TRN2",
        target_bir_lowering=False,
        debug=False,
        enable_asserts=False,
        num_devices=N_CORES,
    )

    d_c1bT = nc.dram_tensor("c1bT", [D, NI], BF16, kind="ExternalInput").ap()
    d_c2bT = nc.dram_tensor("c2bT", [D, N2], BF16, kind="ExternalInput").ap()
    d_sq2neg = nc.dram_tensor("sq2neg", [P, NJT], F32, kind="ExternalInput").ap()

    d_zfin = nc.dram_tensor("zfin", [P, NI], BF16, kind="ExternalOutput").ap()

    with tile.TileContext(nc) as tc, ExitStack() as ctx:
        const = ctx.enter_context(tc.tile_pool(name="const", bufs=1))
        c2pool = ctx.enter_context(tc.tile_pool(name="c2pool", bufs=len(CHUNK_JT)))
        zring = ctx.enter_context(tc.tile_pool(name="zring", bufs=4))
        psumc = ctx.enter_context(tc.tile_pool(name="psumc", bufs=3, space="PSUM"))
        psumw = ctx.enter_context(tc.tile_pool(name="psumw", bufs=1, space="PSUM"))

        t_c1bT = const.tile([P, NI], BF16)
        t_sq2neg = const.tile([P, NJT], F32)
        t_warm = const.tile([P, P], BF16)
        t_wact = const.tile([P, P], BF16)
        t_zD = const.tile([P, 2, NI], BF16)        # DVE STT ping-pong
        t_zAcc = const.tile([P, 2, FOLD_W, NI], BF16)   # fold-chain ping-pong
        t_zfin = const.tile([P, NI], BF16)

        # ---- input DMAs first (sync + gpsimd queues; ACT/DVE stay clean) ----
        t_c2bT = []
        for ci, (s, n) in enumerate(zip(CHUNK_START, CHUNK_JT)):
            t_c2bT.append(c2pool.tile([P, n, P], BF16, name=f"c2bT{ci}"))
        nc.sync.dma_start(
            t_c2bT[0][:],
            d_c2bT[:, : CHUNK_JT[0] * P].rearrange("k (t p) -> k t p", p=P),
        )
        nc.gpsimd.dma_start(t_c1bT[:], d_c1bT)
        nc.sync.dma_start(t_sq2neg[:], d_sq2neg)
        dma_engs = [nc.gpsimd, nc.sync]
        for ci in range(1, len(CHUNK_JT)):
            s, n = CHUNK_START[ci], CHUNK_JT[ci]
            dma_engs[ci % 2].dma_start(
                t_c2bT[ci][:],
                d_c2bT[:, s * P : (s + n) * P].rearrange("k (t p) -> k t p", p=P),
            )

        # accumulator init + PE warm operand (keep DVE free of memsets)
        nc.vector.memset(t_warm[:], 0.0)
        nc.gpsimd.memset(t_zD[:, 0], NEG_BIG)
        nc.gpsimd.memset(t_zAcc[:, 0], NEG_BIG)

        # warm the ACT table before the first drain needs it
        nc.scalar.activation(t_wact[:], t_warm[:],
                             mybir.ActivationFunctionType.Identity, bias=0.0)

        # PE warm-up: start the p-state ramp while inputs stream in
        pw = psumw.tile([P, 512], F32)
        for _ in range(N_WARM):
            nc.tensor.matmul(pw[:, :P], t_warm[:], t_warm[:],
                             start=True, stop=True)

        # ---- cross matmuls (j on partitions) + two-engine drain ----
        nd = nacc = 0
        zslot = 0
        zt = None
        for t in range(NJT):
            ci, ck = TILE_CHUNK[t]
            lhsT = t_c2bT[ci][:, ck]
            pt = psumc.tile([P, NI], F32, name="pcross")
            nc.tensor.matmul(pt[:, :512], lhsT, t_c1bT[:, :512],
                             start=True, stop=True)
            nc.tensor.matmul(pt[:, 512:], lhsT, t_c1bT[:, 512:],
                             start=True, stop=True)
            # fillers: keep PE continuously busy (same stationary weights,
            # scratch bank) so the tensor engine holds its fast p-state
            for w in FILLER_COLS:
                nc.tensor.matmul(pw[:, :w], lhsT, t_c1bT[:, :w],
                                 start=True, stop=True)
            bias = t_sq2neg[:, t : t + 1]
            if t in DVE_TILES:
                nc.vector.scalar_tensor_tensor(
                    out=t_zD[:, (nd + 1) % 2],
                    in0=pt[:],
                    scalar=bias,
                    in1=t_zD[:, nd % 2],
                    op0=mybir.AluOpType.add,
                    op1=mybir.AluOpType.max,
                )
                nd += 1
                if t == 60:
                    # ACT + folds are done: pre-combine zAcc while the
                    # trailing STTs run
                    nc.vector.tensor_max(t_zfin[:], t_zAcc[:, nacc % 2, 0],
                                         t_zAcc[:, nacc % 2, 1])
                    nc.vector.tensor_max(t_zfin[:], t_zfin[:],
                                         t_zAcc[:, nacc % 2, 2])
                    nc.vector.tensor_max(t_zfin[:], t_zfin[:],
                                         t_zAcc[:, nacc % 2, 3])
            else:
                if zslot == 0:
                    zt = zring.tile([P, FOLD_W, NI], BF16, name="zt")
                nc.scalar.activation(
                    zt[:, zslot], pt[:], mybir.ActivationFunctionType.Identity,
                    bias=bias, scale=1.0,
                )
                zslot += 1
                if zslot == FOLD_W:
                    nc.vector.tensor_max(t_zAcc[:, (nacc + 1) % 2],
                                         t_zAcc[:, nacc % 2], zt[:])
                    nacc += 1
                    zslot = 0

        # ---- final: fold in zD + ship out on two queues ----
        nc.vector.tensor_max(t_zfin[:], t_zfin[:], t_zD[:, nd % 2])
        nc.sync.dma_start(d_zfin[:, : NI // 2], t_zfin[:, : NI // 2])
        nc.gpsimd.dma_start(d_zfin[:, NI // 2 :], t_zfin[:, NI // 2 :])

    nc.compile()
    return nc


def _prep_inputs(cluster1: np.ndarray, cluster2: np.ndarray):
    """Host-side sharding + operand layout prep."""
    c2b = cluster2.astype(BF16_NP)
    c2bT = np.ascontiguousarray(c2b.T)                       # [128, 8192] bf16
    sq2 = (c2b.astype(np.float32) ** 2).sum(axis=1)          # [8192] fp32
    sq2neg = np.ascontiguousarray((-sq2).reshape(NJT, P).T).astype(np.float32)

    in_maps = []
    for c in range(N_CORES):
        c1s = cluster1[c * NI : (c + 1) * NI]
        c1bT = np.ascontiguousarray((2.0 * c1s).astype(BF16_NP).T)  # [128, 1024]
        in_maps.append({
            "c1bT": c1bT,
            "c2bT": c2bT,
            "sq2neg": sq2neg,
        })
    return in_maps


def _finish(results, cluster1, cluster2) -> np.float32:
    """Host: partition-max of the per-core partials + the O(N*D) stats."""
    c1 = np.asarray(cluster1, np.float32)
    c2 = np.asarray(cluster2, np.float32)
    dist_sum = 0.0
    for c, r in enumerate(results):
        z = np.asarray(r["zfin"], np.float32)   # [128 j-lane, 1024 i]
        gmax = z.max(axis=0)                    # [1024] max_j (2<c1,c2> - |c2|^2)
        c1s = c1[c * NI : (c + 1) * NI].astype(np.float64)
        sq1 = (c1s ** 2).sum(axis=1)            # [1024]
        dist_sum += (sq1 - gmax.astype(np.float64)).sum()
    dist = dist_sum / N1

    m1 = c1.mean(axis=0, dtype=np.float64)
    m2 = c2.mean(axis=0, dtype=np.float64)
    mean_loss = ((m1 - m2) ** 2).mean()
    q1 = (c1.astype(np.float64) ** 2).mean(axis=0)
    var = q1 - m1 ** 2
    disp = np.maximum(MIN_VARIANCE - var, 0.0).mean()
    return np.float32(mean_loss + dist + disp)


def _run(inputs, trace=False, **kwargs):
    """Run on the 8 NeuronCores. Returns (loss_scalar, BassKernelResults)."""
    if "nc" not in _cached:
        _cached["nc"] = _build_program()
    nc = _cached["nc"]
    c1 = np.asarray(inputs["cluster1"], np.float32)
    c2 = np.asarray(inputs["cluster2"], np.float32)
    in_maps = _prep_inputs(c1, c2)
    res = run_bass_kernel_spmd(nc, in_maps, list(range(N_CORES)), trace=trace,
                               **kwargs)
    loss = _finish(res.results, c1, c2)
    return loss, res


def kernel(cluster1: np.ndarray, cluster2: np.ndarray) -> np.ndarray:
    loss, _ = _run({"cluster1": cluster1, "cluster2": cluster2})
    return np.asarray(loss, dtype=np.float32)


# revision 11
# speedup vs baseline: 1.1334x; 1.0924x over previous
"""Trainium2 Bass kernel for nn_CustomLoss_45449343926664 (retrieval_knn).

loss = mse(mean(c1), mean(c2))
     + mean_i min_j ||c1_i - c2_j||^2
     + mean_k relu(0.1 - var(c1)_k)

Device does ONLY the O(N^2) part: each core computes its [1024, 8192]
block of 2<c1_i, c2_j> on the PE (bf16, j-on-partitions: psum tile
[128 j, 1024 i] per j-tile) and row-maxes z = 2<c1,c2> - |c2_j|^2 with a
two-engine drain (each psum element passes exactly once through DVE or
ACT, the only engines with a PSUM read port; both read PSUM at 1
elem/lane/cycle, which is the hard wall of this kernel):

  - DVE tiles (20): fused scalar_tensor_tensor drain
        zD' = max(psum + bias, zD)     (ping-pong accum)
  - ACT tiles (44): activation(Identity, bias) -> bf16 z pairs, folded
    into the zAcc running max by one [128,2,1024] bf16 tensor_max (2x
    mode) on DVE per pair.

Filler matmuls into a scratch psum bank keep the PE busy fraction high
so its p-state stays fast.  The final partial-max tensor zfin
([128, 1024] bf16 per core) is DMA'd out in two halves on separate
queues; the host does the partition-max of zfin, |c1_i|^2, the
means/variances, and the scalar combine (all O(N*D))."""
import os
import sys

import numpy as np
import ml_dtypes

if os.path.isdir("/opt/trn_rl_repo") and "/opt/trn_rl_repo" not in sys.path:
    sys.path.insert(0, "/opt/trn_rl_repo")

from contextlib import ExitStack

import concourse.bass as bass
import concourse.tile as tile
from concourse import bacc, mybir
from concourse.bass_utils import run_bass_kernel_spmd

F32 = mybir.dt.float32
BF16 = mybir.dt.bfloat16
BF16_NP = ml_dtypes.bfloat16
NEG_BIG = -3.0e38

N_CORES = 8
N1 = 8192            # cluster1 rows (total)
N2 = 8192            # cluster2 rows
D = 128              # feature dim = partition count
P = 128
NI = N1 // N_CORES   # 1024 c1 rows per core
NJT = N2 // P        # 64 j-tiles of 128

# c2bT DMA chunk sizes in j-tiles (first small so matmuls start early)
CHUNK_JT = [2, 6, 8, 8, 8, 8, 8, 8, 8]
CHUNK_START = [0, 2, 8, 16, 24, 32, 40, 48, 56]
TILE_CHUNK = {}
for _ci, (_s, _n) in enumerate(zip(CHUNK_START, CHUNK_JT)):
    for _k in range(_n):
        TILE_CHUNK[_s + _k] = (_ci, _k)

# 20 tiles drained by the fused DVE path; the other 44 go to ACT (whose
# clock is faster) and are folded on DVE in bf16 2x mode.
DVE_TILES = {t for t in range(NJT) if t % 16 in (2, 5, 8, 11, 14)}
N_WARM = 16
FILLER_COLS = [512, 128]     # filler matmul widths per j-tile
MIN_VARIANCE = 0.1

_cached = {}


def _build_program():
    """Build + compile the single-core SPMD program (same for all cores)."""
    nc = bacc.Bacc(
        "TRN2",
        target_bir_lowering=False,
        debug=False,
        enable_asserts=False,
        num_devices=N_CORES,
    )

    d_c1bT = nc.dram_tensor("c1bT", [D, NI], BF16, kind="ExternalInput").ap()
    d_c2bT = nc.dram_tensor("c2bT", [D, N2], BF16, kind="ExternalInput").ap()
    d_sq2neg = nc.dram_tensor("sq2neg", [P, NJT], F32, kind="ExternalInput").ap()

    d_zfin = nc.dram_tensor("zfin", [P, NI], BF16, kind="ExternalOutput").ap()

    with tile.TileContext(nc) as tc, ExitStack() as ctx:
        const = ctx.enter_context(tc.tile_pool(name="const", bufs=1))
        c2pool = ctx.enter_context(tc.tile_pool(name="c2pool", bufs=len(CHUNK_JT)))
        zring = ctx.enter_context(tc.tile_pool(name="zring", bufs=6))
        psumc = ctx.enter_context(tc.tile_pool(name="psumc", bufs=3, space="PSUM"))
        psumw = ctx.enter_context(tc.tile_pool(name="psumw", bufs=1, space="PSUM"))

        t_c1bT = const.tile([P, NI], BF16)
        t_sq2neg = const.tile([P, NJT], F32)
        t_warm = const.tile([P, P], BF16)
        t_wact = const.tile([P, P], BF16)
        t_zD = const.tile([P, 2, NI], BF16)        # DVE STT ping-pong
        t_zAcc = const.tile([P, 2, 2, NI], BF16)   # fold-chain ping-pong
        t_zfin = const.tile([P, NI], BF16)

        # ---- input DMAs first (sync + gpsimd queues; ACT/DVE stay clean) ----
        t_c2bT = []
        for ci, (s, n) in enumerate(zip(CHUNK_START, CHUNK_JT)):
            t_c2bT.append(c2pool.tile([P, n, P], BF16, name=f"c2bT{ci}"))
        nc.sync.dma_start(
            t_c2bT[0][:],
            d_c2bT[:, : CHUNK_JT[0] * P].rearrange("k (t p) -> k t p", p=P),
        )
        nc.gpsimd.dma_start(t_c1bT[:], d_c1bT)
        nc.sync.dma_start(t_sq2neg[:], d_sq2neg)
        dma_engs = [nc.gpsimd, nc.sync]
        for ci in range(1, len(CHUNK_JT)):
            s, n = CHUNK_START[ci], CHUNK_JT[ci]
            dma_engs[ci % 2].dma_start(
                t_c2bT[ci][:],
                d_c2bT[:, s * P : (s + n) * P].rearrange("k (t p) -> k t p", p=P),
            )

        # accumulator init + PE warm operand (keep DVE free of memsets)
        nc.vector.memset(t_warm[:], 0.0)
        nc.gpsimd.memset(t_zD[:, 0], NEG_BIG)
        nc.gpsimd.memset(t_zAcc[:, 0], NEG_BIG)

        # warm the ACT table before the first drain needs it
        nc.scalar.activation(t_wact[:], t_warm[:],
                             mybir.ActivationFunctionType.Identity, bias=0.0)

        # PE warm-up: start the p-state ramp while inputs stream in
        pw = psumw.tile([P, 512], F32)
        for _ in range(N_WARM):
            nc.tensor.matmul(pw[:, :P], t_warm[:], t_warm[:],
                             start=True, stop=True)

        # ---- cross matmuls (j on partitions) + two-engine drain ----
        nd = nacc = 0
        zhalf = 0
        zt = None
        for t in range(NJT):
            ci, ck = TILE_CHUNK[t]
            lhsT = t_c2bT[ci][:, ck]
            pt = psumc.tile([P, NI], F32, name="pcross")
            nc.tensor.matmul(pt[:, :512], lhsT, t_c1bT[:, :512],
                             start=True, stop=True)
            nc.tensor.matmul(pt[:, 512:], lhsT, t_c1bT[:, 512:],
                             start=True, stop=True)
            # fillers: keep PE continuously busy (same stationary weights,
            # scratch bank) so the tensor engine holds its fast p-state
            for w in FILLER_COLS:
                nc.tensor.matmul(pw[:, :w], lhsT, t_c1bT[:, :w],
                                 start=True, stop=True)
            bias = t_sq2neg[:, t : t + 1]
            if t in DVE_TILES:
                nc.vector.scalar_tensor_tensor(
                    out=t_zD[:, (nd + 1) % 2],
                    in0=pt[:],
                    scalar=bias,
                    in1=t_zD[:, nd % 2],
                    op0=mybir.AluOpType.add,
                    op1=mybir.AluOpType.max,
                )
                nd += 1
            else:
                if zhalf == 0:
                    zt = zring.tile([P, 2, NI], BF16, name="zt")
                nc.scalar.activation(
                    zt[:, zhalf], pt[:], mybir.ActivationFunctionType.Identity,
                    bias=bias, scale=1.0,
                )
                if zhalf == 1:
                    nc.vector.tensor_max(t_zAcc[:, (nacc + 1) % 2],
                                         t_zAcc[:, nacc % 2], zt[:])
                    nacc += 1
                zhalf ^= 1
        if zhalf == 1:   # lone trailing ACT tile: pad its pair-half
            nc.gpsimd.memset(zt[:, 1], NEG_BIG)
            nc.vector.tensor_max(t_zAcc[:, (nacc + 1) % 2],
                                 t_zAcc[:, nacc % 2], zt[:])
            nacc += 1

        # ---- final: combine accumulators + ship out on two queues ----
        nc.vector.tensor_max(t_zfin[:], t_zAcc[:, nacc % 2, 0],
                             t_zAcc[:, nacc % 2, 1])
        nc.vector.tensor_max(t_zfin[:], t_zfin[:], t_zD[:, nd % 2])
        nc.sync.dma_start(d_zfin[:, : NI // 2], t_zfin[:, : NI // 2])
        nc.gpsimd.dma_start(d_zfin[:, NI // 2 :], t_zfin[:, NI // 2 :])

    nc.compile()
    return nc


def _prep_inputs(cluster1: np.ndarray, cluster2: np.ndarray):
    """Host-side sharding + operand layout prep."""
    c2b = cluster2.astype(BF16_NP)
    c2bT = np.ascontiguousarray(c2b.T)                       # [128, 8192] bf16
    sq2 = (c2b.astype(np.float32) ** 2).sum(axis=1)          # [8192] fp32
    sq2neg = np.ascontiguousarray((-sq2).reshape(NJT, P).T).astype(np.float32)

    in_maps = []
    for c in range(N_CORES):
        c1s = cluster1[c * NI : (c + 1) * NI]
        c1bT = np.ascontiguousarray((2.0 * c1s).astype(BF16_NP).T)  # [128, 1024]
        in_maps.append({
            "c1bT": c1bT,
            "c2bT": c2bT,
            "sq2neg": sq2neg,
        })
    return in_maps


def _finish(results, cluster1, cluster2) -> np.float32:
    """Host: partition-max of the per-core partials + the O(N*D) stats."""
    c1 = np.asarray(cluster1, np.float32)
    c2 = np.asarray(cluster2, np.float32)
    dist_sum = 0.0
    for c, r in enumerate(results):
        z = np.asarray(r["zfin"], np.float32)   # [128 j-lane, 1024 i]
        gmax = z.max(axis=0)                    # [1024] max_j (2<c1,c2> - |c2|^2)
        c1s = c1[c * NI : (c + 1) * NI].astype(np.float64)
        sq1 = (c1s ** 2).sum(axis=1)            # [1024]
        dist_sum += (sq1 - gmax.astype(np.float64)).sum()
    dist = dist_sum / N1

    m1 = c1.mean(axis=0, dtype=np.float64)
    m2 = c2.mean(axis=0, dtype=np.float64)
    mean_loss = ((m1 - m2) ** 2).mean()
    q1 = (c1.astype(np.float64) ** 2).mean(axis=0)
    var = q1 - m1 ** 2
    disp = np.maximum(MIN_VARIANCE - var, 0.0).mean()
    return np.float32(mean_loss + dist + disp)


def _run(inputs, trace=False, **kwargs):
    """Run on the 8 NeuronCores. Returns (loss_scalar, BassKernelResults)."""
    if "nc" not in _cached:
        _cached["nc"] = _build_program()
    nc = _cached["nc"]
    c1 = np.asarray(inputs["cluster1"], np.float32)
    c2 = np.asarray(inputs["cluster2"], np.float32)
    in_maps = _prep_inputs(c1, c2)
    res = run_bass_kernel_spmd(nc, in_maps, list(range(N_CORES)), trace=trace,
                               **kwargs)
    loss = _finish(res.results, c1, c2)
    return loss, res


def kernel(cluster1: np.ndarray, cluster2: np.ndarray) -> np.ndarray:
    loss, _ = _run({"cluster1": cluster1, "cluster2": cluster2})
    return np.asarray(loss, dtype=np.float32)
